# revision 8
# baseline (speedup 1.0000x reference)
"""HadamardTest kernel for Trainium2, 8-core data-parallel SPMD.

out[n, c] = (z_re @ refT)^2 + (z_im @ refT)^2, ref = L2-normalized zero-padded canon.

Sharding: z_re/z_im split along samples into 8 shards of 2048 rows; the tiny
normalized ref table is replicated. Each core computes its [10, 2048] slice
of the (transposed) output.

The kernel is HBM-bound (measured pure-DMA floor ~9.0us/iter at ~360 GB/s
per core), so the design halves input bytes and keeps every other engine off
the critical path:

  - z is quantized host-side to float8e3 (e3m4: 4 mantissa bits) scaled by
    64 into the format's normal range; ref is bf16 scaled by 1/64 so PSUM
    accumulates the unscaled overlap s directly. e3m4 z x bf16 ref gives
    rel err ~1.4e-2 (gate 2e-2) at half the HBM traffic of bf16.
    (e4m3 z measures 4.3e-2 - fails; bf16 z passes at 2.5e-3 but is 2x the
    bytes and ~2x slower.)
  - host pre-transposes z into a [128, 6*4096] table (contraction chunk c
    of 128 dims at column block c; within a block 2048 re cols then 2048
    im cols): 2 DMA loads of 12KB/partition, all 128 partitions -> all 16
    SDMA engines balanced. (112-row layouts measure ~30% slower; a
    16-partition tail DMA concentrates on 2 engines - also slower; 2 loads
    beat 3 and 1 - fewer per-copy semaphore waits but still pipelined.)
  - the 16-dim contraction tail (784 = 6*128 + 16) is packed over all 128
    partitions as a [128, 512] block appended to the z table (no padding,
    no extra dma_start -> 2 loads/copy total): row-group q holds moving-col
    block g=2q in rows 32q..32q+16 and block 2q+1 in rows 32q+16..32q+32;
    the two tail weight blocks are zero in the opposite half so each
    matmul contracts all 32 rows but only its block's data contributes.
  - the 10-wide output would waste 118 of 128 PE columns in one matmul, so
    the 4 sample blocks of 512 go to the 4 32-column PE groups via
    tile_position=(0,32j) and run concurrently (PE off critical path);
    all 4 blocks of re accumulate in ONE PSUM bank, im in a second. The
    tail adds 8 row+col-tiled matmuls at (32q, 32j). Ref chunks are
    zero-padded 10->32 cols so every PSUM partition is written.
  - ACT squares re/im PSUM->SBUF full-width, DVE adds into an f16 tile,
    4 small f16 DMAs on the scalar-queue HWDGE store the [10, 512] slices
    (f16 out measures ~0.6us faster than f32; host upcasts).
  - repeat mode unrolls 64 copies per For_i trip: the trip boundary
    serializes engines; deep unrolling amortizes that cost.
"""
import numpy as np
import ml_dtypes

import concourse.mybir as mybir
import concourse.tile as tile
from concourse import bacc
from concourse.bass_utils import run_bass_kernel_spmd

F32 = mybir.dt.float32
F16 = mybir.dt.float16
BF16 = mybir.dt.bfloat16
E3M4 = mybir.dt.float8e3
BF16NP = ml_dtypes.bfloat16
E3M4NP = ml_dtypes.float8_e3m4

N = 16384          # total samples
DIM = 1024         # state dimension (2**10)
C = 10             # classes
IMG = 784          # 28*28 pixels before zero-pad
N_CORES = 8
NS = N // N_CORES  # 2048 samples per core
NT = 512           # matmul moving-block (one PSUM bank of fp32)
NB = NS // NT      # 4 sample blocks per core = 4 PE column groups
P = 128
M = 32             # stationary cols per group (10 classes + 22 zero pad)
KF = IMG // P      # 6 full contraction chunks
KT = IMG - KF * P  # 16-row tail
KCH = KF + 1
ZSCALE = 64.0      # host scale into e3m4 range; ref carries 1/64
UNROLL = 64

_CACHE = {}


def build_kernel(repeat=None):
    key = ("nc", repeat)
    if key in _CACHE:
        return _CACHE[key]
    nc = bacc.Bacc(None, target_bir_lowering=False, debug=False,
                   num_devices=N_CORES)
    # zdr[p, c*4096 + n] = chunk c, contraction row p, moving col n;
    # the packed tail block [128, 512] is appended at column 6*4096
    zdr_d = nc.dram_tensor("zdr", [P, KF * 2 * NS + NT], E3M4,
                           kind="ExternalInput").ap()
    ref_d = nc.dram_tensor("refp", [P, (KCH + 1) * M], BF16,
                           kind="ExternalInput").ap()
    outT_d = nc.dram_tensor("outT", [C, NS], F16, kind="ExternalOutput").ap()

    with tile.TileContext(nc) as tc:
        with (
            tc.tile_pool(name="const", bufs=1) as cpool,
            tc.tile_pool(name="zload", bufs=2) as zpool,
            tc.tile_pool(name="eps", bufs=4) as epool,
            tc.tile_pool(name="outsb", bufs=2) as opool,
            tc.tile_pool(name="ps", bufs=2, space="PSUM") as pspool,
        ):
            rt = cpool.tile([P, (KCH + 1) * M], BF16)
            nc.sync.dma_start(out=rt[:], in_=ref_d[:])

            def body():
                # 2 z DMAs: chunks 0-2, then chunks 3-5 + packed tail
                zts = []
                for b in range(2):
                    W = 3 * 2 * NS + (NT if b == 1 else 0)
                    zt = zpool.tile([P, W], E3M4, tag=f"z{b}",
                                    name=f"zt{b}")
                    nc.sync.dma_start(
                        out=zt[:],
                        in_=zdr_d[:, b * 3 * 2 * NS:b * 3 * 2 * NS + W])
                    zts.append(zt)
                ztl, tlo = zts[1], 3 * 2 * NS

                pre = pspool.tile([P, NT], F32, tag="pre", name="pre")
                pim = pspool.tile([P, NT], F32, tag="pim", name="pim")
                for k in range(KF):
                    zsrc, off = zts[k // 3], (k % 3) * 2 * NS
                    wk = rt[:, k * M:(k + 1) * M]
                    for j in range(NB):
                        nc.tensor.matmul(
                            pre[M * j:M * (j + 1), :], wk,
                            zsrc[:, off + j * NT:off + (j + 1) * NT],
                            start=(k == 0), stop=False,
                            tile_position=(0, M * j))
                    for j in range(NB):
                        nc.tensor.matmul(
                            pim[M * j:M * (j + 1), :], wk,
                            zsrc[:, off + NS + j * NT:off + NS + (j + 1) * NT],
                            start=(k == 0), stop=False,
                            tile_position=(0, M * j))
                # tail: block g=2q+half in rows 32q+16*half..; weight
                # block (KF+half) is zero in the other half; output col
                # group j = g%4 for re (g<4) / im (g>=4)
                for g in range(2 * NB):
                    q, half = g // 2, g % 2
                    ps, j = (pre, g) if g < NB else (pim, g - NB)
                    wb = (KF + half) * M
                    nc.tensor.matmul(
                        ps[M * j:M * (j + 1), :],
                        rt[32 * q:32 * (q + 1), wb:wb + M],
                        ztl[32 * q:32 * (q + 1), tlo:tlo + NT],
                        start=False, stop=True,
                        tile_position=(32 * q, M * j))

                tre = epool.tile([P, NT], F32, tag="tre", name="tre")
                tim = epool.tile([P, NT], F32, tag="tim", name="tim")
                osb = opool.tile([P, NT], F16, tag="osb", name="osb")
                nc.scalar.activation(
                    out=tre[:], in_=pre[:],
                    func=mybir.ActivationFunctionType.Square)
                nc.scalar.activation(
                    out=tim[:], in_=pim[:],
                    func=mybir.ActivationFunctionType.Square)
                nc.vector.tensor_add(out=osb[:], in0=tre[:], in1=tim[:])
                # out-DMA on ACT (the other HWDGE engine) to keep the sync
                # sequencer free for the next copy's z loads.
                for j in range(NB):
                    nc.scalar.dma_start(
                        out=outT_d[:, j * NT:(j + 1) * NT],
                        in_=osb[M * j:M * j + C, :])

            if repeat is None:
                body()
            else:
                # Unroll 64 body copies per hardware-loop trip: the For_i
                # boundary serializes engines, so cross-copy overlap (DMA of
                # copy b+1 under matmuls of copy b) only happens within a
                # trip; deep unrolling measures ~0.5us/iter faster than 8.
                trips, rem = divmod(repeat, UNROLL)
                assert rem == 0, f"repeat must be a multiple of {UNROLL}"
                with tc.For_i(0, trips, 1,
                              hint_engines=(mybir.EngineType.PE,)):
                    for _ in range(UNROLL):
                        body()

    nc.finalize()
    _CACHE[key] = nc
    return nc


def prepare_in_maps(z_re, z_im, canon):
    ref = np.asarray(canon, dtype=np.float32).reshape(C, IMG)
    ref = ref / np.linalg.norm(ref, axis=1, keepdims=True)
    ref = ref / ZSCALE
    # packed stationary chunks: refp[p, k*M + c] = ref[c, k*128 + p], cols
    # C..M-1 zero so every PSUM partition is written (no stale reads).
    # Tail chunk (block KF) is replicated into each 32-partition row group.
    refp = np.zeros((P, (KCH + 1) * M), dtype=np.float32)
    for k in range(KF):
        refp[:, k * M:k * M + C] = ref[:, k * P:(k + 1) * P].T
    for q in range(4):
        refp[32 * q:32 * q + KT, KF * M:KF * M + C] = ref[:, KF * P:IMG].T
        refp[32 * q + KT:32 * (q + 1),
             (KF + 1) * M:(KF + 1) * M + C] = ref[:, KF * P:IMG].T
    refp = refp.astype(BF16NP)

    def quant(z):
        z = np.asarray(z, dtype=np.float32)[:, :IMG] * ZSCALE
        return np.clip(z, -15.5, 15.5).astype(E3M4NP)

    zre8, zim8 = quant(z_re), quant(z_im)
    in_maps = []
    for c in range(N_CORES):
        s = slice(c * NS, (c + 1) * NS)
        zT = np.concatenate([zre8[s].T, zim8[s].T], axis=1)  # [IMG, 2*NS]
        zdr = np.ascontiguousarray(
            zT[:KF * P].reshape(KF, P, 2 * NS).transpose(1, 0, 2)
            .reshape(P, KF * 2 * NS))
        tl = zT[KF * P:IMG]                                  # [16, 4096]
        tsp = np.zeros((P, NT), E3M4NP)
        for g in range(2 * NB):
            q, half = g // 2, g % 2
            tsp[32 * q + half * KT:32 * q + (half + 1) * KT, :] = \
                tl[:, g * NT:(g + 1) * NT]
        zdr = np.ascontiguousarray(np.concatenate([zdr, tsp], axis=1))
        in_maps.append({"zdr": zdr, "refp": refp})
    return in_maps


def kernel(z_re, z_im, canon):
    nc = build_kernel()
    in_maps = prepare_in_maps(z_re, z_im, canon)
    res = run_bass_kernel_spmd(nc, in_maps, list(range(N_CORES)), trace=False)
    out = np.empty((N, C), dtype=np.float32)
    for c in range(N_CORES):
        out[c * NS:(c + 1) * NS] = res.results[c]["outT"].T.astype(np.float32)
    return out
